# revision 1
# baseline (speedup 1.0000x reference)
"""NT-Xent contrastive loss on 8 Trainium2 NeuronCores (Bass/Tile).

Strategy (no collectives -- measured ncfw latency floor ~85us makes the
all-gather hint design strictly worse):
  * Host pre-transposes embedded_data to embT [2048, 8192] (pure layout).
  * Slab cover: core c loads the 4 row-slabs S_c = {c, c+1, c+2, c+4} (mod 8)
    of emb (32 MiB/core). Every slab PAIR meets on some core (Z8 difference
    cover: slot-pairs at differences 1,2,3,4), so each of the 36 distinct
    1024x1024 blocks of the 8192x8192 similarity matrix is computed once
    globally; block (i,j) yields exp-row-sums for slab i (ACT accum) AND
    exp-col-sums for slab j (ones-matmul), exploiting sim symmetry.
  * Per core, uniform SPMD program: head matmul out_headT = W.T @ embT_slab
    (fp32r, 1 cyc/row), L2 normalize via ones-matmul normsq + Sqrt +
    reciprocal + K=1 broadcast matmul, then 5 sim blocks (diag + 4 pairs):
    psum [128,1024] fp32 -> ACT exp(10*x) with fused row-sum accum ->
    f32r exp tile -> ones-matmul col-sums. Diagonal exp values extracted
    exactly via a shifted-identity mask (mult+reduce) and subtracted on host.
  * pos term: elementwise product of slabs c and c+4 + ones-matmul -> the
    positive-pair similarities; log(pos) = 10*possim exactly (no exp needed).
  * Host (fp64): sums partial row/col contributions, subtracts diag,
    loss = -mean(10*possim - log(neg)).
"""
import numpy as np

SLOTS = [(c, (c + 1) % 8, (c + 2) % 8, (c + 4) % 8) for c in range(8)]
# blocks in local slot coords: (stationary, moving). B0 = diag.
BLOCKS = [(0, 0), (0, 1), (0, 2), (1, 3), (0, 3)]

_CACHE = {}


def _build():
    if "nc" in _CACHE:
        return _CACHE["nc"]
    import concourse.bacc as bacc
    import concourse.tile as tile
    import concourse.mybir as mybir

    F32, F32R = mybir.dt.float32, mybir.dt.float32r
    AF = mybir.ActivationFunctionType
    ALU = mybir.AluOpType

    nc = bacc.Bacc("TRN2", num_devices=8, debug=False)
    a_emb = nc.dram_tensor("embT", [2048, 4096], F32, kind="ExternalInput").ap()
    a_W = nc.dram_tensor("W", [2048, 256], F32, kind="ExternalInput").ap()
    a_b = nc.dram_tensor("b", [256], F32, kind="ExternalInput").ap()
    a_ones = nc.dram_tensor("ones", [128, 128], F32, kind="ExternalInput").ap()
    a_mask = nc.dram_tensor("mask", [128, 2048], F32, kind="ExternalInput").ap()
    o_rp = nc.dram_tensor("rowpart", [5, 1024], F32, kind="ExternalOutput").ap()
    o_cp = nc.dram_tensor("colpart", [4, 1024], F32, kind="ExternalOutput").ap()
    o_dg = nc.dram_tensor("diagexp", [1, 1024], F32, kind="ExternalOutput").ap()
    o_ps = nc.dram_tensor("possim", [1, 1024], F32, kind="ExternalOutput").ap()

    with tile.TileContext(nc) as tc:
        with tc.tile_pool(name="sb", bufs=1) as sb, \
             tc.tile_pool(name="emb", bufs=10) as embp, \
             tc.tile_pool(name="work", bufs=2) as wk, \
             tc.tile_pool(name="expp", bufs=3) as expp, \
             tc.tile_pool(name="headp", bufs=1, space="PSUM") as headp, \
             tc.tile_pool(name="simp", bufs=2, space="PSUM") as simp, \
             tc.tile_pool(name="csp", bufs=2, space="PSUM") as csp:

            t_W = sb.tile([128, 16, 256], F32R, name="t_W")
            nc.sync.dma_start(t_W[:], a_W.bitcast(F32R).rearrange("(kc p) d -> p kc d", p=128))
            t_b = sb.tile([128, 2], F32, name="t_b")
            nc.sync.dma_start(t_b[:], a_b.rearrange("(dh p) -> p dh", p=128))
            ones_col = sb.tile([128, 1], F32R, name="ones_col")
            nc.sync.dma_start(ones_col[:], a_ones.bitcast(F32R)[:, 0:1])
            ones_row = sb.tile([1, 128], F32, name="ones_row")
            nc.sync.dma_start(ones_row[:], a_ones[0:1, :])
            t_mask = sb.tile([128, 2048], F32, name="t_mask")
            nc.sync.dma_start(t_mask[:], a_mask[:])

            # staging accumulators
            rp_st = sb.tile([128, 5, 8], F32, name="rp_st")
            dg_st = sb.tile([128, 8], F32, name="dg_st")
            cp_st = sb.tile([1, 4096], F32, name="cp_st")
            ps_st = sb.tile([1, 1024], F32, name="ps_st")

            t_on = [sb.tile([128, 2, 1024], F32R, name=f"t_on{k}") for k in range(4)]

            def stage_a(k):
                t_h = wk.tile([128, 2, 1024], F32, name="t_h", tag="th")
                for h in range(2):
                    tes = []
                    for g in range(8):
                        t_e = embp.tile([128, 2, 512], F32R, name="t_e", tag="emb")
                        src = a_emb.bitcast(F32R)[256 * g:256 * (g + 1),
                                                  1024 * k + 512 * h:1024 * k + 512 * (h + 1)]
                        nc.sync.dma_start(t_e[:], src.rearrange("(c p) r -> p c r", p=128))
                        tes.append(t_e)
                    p_h = headp.tile([128, 2, 512], F32, name="p_h", tag="head")
                    for g in range(8):
                        for cc in range(2):
                            kk = 2 * g + cc
                            for dh in range(2):
                                nc.tensor.matmul(
                                    p_h[:, dh, :],
                                    t_W[:, kk, dh * 128:(dh + 1) * 128],
                                    tes[g][:, cc, :],
                                    start=(kk == 0), stop=(kk == 15),
                                )
                    for dh in range(2):
                        nc.vector.tensor_scalar_add(
                            t_h[:, dh, 512 * h:512 * (h + 1)], p_h[:, dh, :],
                            t_b[:, dh:dh + 1])
                t_sq = wk.tile([128, 2, 1024], F32R, name="t_sq", tag="sq")
                nc.vector.tensor_tensor(t_sq[:], t_h[:], t_h[:], ALU.mult)
                p_ns = [csp.tile([1, 512], F32, name=f"p_ns{nb}", tag="cs") for nb in range(2)]
                for nb in range(2):
                    for dh in range(2):
                        nc.tensor.matmul(p_ns[nb][:], ones_col[:],
                                         t_sq[:, dh, 512 * nb:512 * (nb + 1)],
                                         start=(dh == 0), stop=(dh == 1))
                t_nrm = wk.tile([1, 1024], F32, name="t_nrm", tag="nrm")
                for nb in range(2):
                    nc.scalar.activation(t_nrm[:, 512 * nb:512 * (nb + 1)], p_ns[nb][:], AF.Sqrt)
                t_ri = wk.tile([1, 1024], F32, name="t_ri", tag="ri")
                nc.vector.reciprocal(t_ri[:], t_nrm[:])
                p_bc = headp.tile([128, 2, 512], F32, name="p_bc", tag="head")
                for nb in range(2):
                    nc.tensor.matmul(p_bc[:, nb, :], ones_row[:],
                                     t_ri[:, 512 * nb:512 * (nb + 1)], start=True, stop=True)
                bc_flat = p_bc[:].rearrange("p a b -> p (a b)")
                for dh in range(2):
                    nc.vector.tensor_tensor(t_on[k][:, dh, :], t_h[:, dh, :], bc_flat, ALU.mult)

            def block(bslot, a, bm):
                p_cs = None
                if bslot > 0:
                    p_cs = [csp.tile([1, 512], F32, name=f"p_cs{bslot}_{nb}", tag="cs")
                            for nb in range(2)]
                for mb in range(8):
                    p_sim = simp.tile([128, 1024], F32, name="p_sim", tag="sim")
                    for dh in range(2):
                        for nb in range(2):
                            nc.tensor.matmul(
                                p_sim[:, 512 * nb:512 * (nb + 1)],
                                t_on[a][:, dh, 128 * mb:128 * (mb + 1)],
                                t_on[bm][:, dh, 512 * nb:512 * (nb + 1)],
                                start=(dh == 0), stop=(dh == 1))
                    t_exp = expp.tile([128, 1024], F32R, name="t_exp", tag="exp")
                    nc.scalar.activation(t_exp[:], p_sim[:], AF.Exp, scale=10.0,
                                         accum_out=rp_st[:, bslot, mb:mb + 1])
                    if bslot > 0:
                        for nb in range(2):
                            nc.tensor.matmul(p_cs[nb][:], ones_col[:],
                                             t_exp[:, 512 * nb:512 * (nb + 1)],
                                             start=(mb == 0), stop=(mb == 7))
                    else:
                        t_sc = expp.tile([128, 1024], F32, name="t_sc", tag="sc")
                        nc.vector.tensor_tensor(
                            t_sc[:], t_exp[:].bitcast(F32),
                            t_mask[:, 1024 - 128 * mb:2048 - 128 * mb], ALU.mult)
                        nc.vector.tensor_reduce(dg_st[:, mb:mb + 1], t_sc[:],
                                                mybir.AxisListType.X, ALU.add)
                if bslot > 0:
                    for nb in range(2):
                        nc.vector.tensor_copy(
                            cp_st[0:1, 1024 * (bslot - 1) + 512 * nb:
                                  1024 * (bslot - 1) + 512 * (nb + 1)], p_cs[nb][:])

            stage_a(0)
            block(0, 0, 0)
            stage_a(1)
            block(1, 0, 1)
            stage_a(2)
            block(2, 0, 2)
            stage_a(3)
            block(3, 1, 3)
            block(4, 0, 3)

            # pos: elementwise product slabs slot0 x slot3, column sums over d
            t_pp = wk.tile([128, 2, 1024], F32R, name="t_pp", tag="sq")
            for dh in range(2):
                nc.vector.tensor_tensor(t_pp[:, dh, :], t_on[0][:, dh, :].bitcast(F32),
                                        t_on[3][:, dh, :].bitcast(F32), ALU.mult)
            p_ps = [csp.tile([1, 512], F32, name=f"p_ps{nb}", tag="cs") for nb in range(2)]
            for nb in range(2):
                for dh in range(2):
                    nc.tensor.matmul(p_ps[nb][:], ones_col[:],
                                     t_pp[:, dh, 512 * nb:512 * (nb + 1)],
                                     start=(dh == 0), stop=(dh == 1))
                nc.vector.tensor_copy(ps_st[0:1, 512 * nb:512 * (nb + 1)], p_ps[nb][:])

            # final DMAs
            for bslot in range(5):
                nc.sync.dma_start(
                    o_rp[bslot:bslot + 1, :].rearrange("o (m p) -> p (o m)", p=128),
                    rp_st[:, bslot, :])
            nc.sync.dma_start(o_dg.rearrange("o (m p) -> p (o m)", p=128), dg_st[:])
            nc.sync.dma_start(o_cp.rearrange("a r -> (a r)")[None, :], cp_st[:])
            nc.sync.dma_start(o_ps[:], ps_st[:])

    nc.compile()
    _CACHE["nc"] = nc
    return nc


def _host_inputs(embedded_data, W, b):
    embT = np.ascontiguousarray(np.asarray(embedded_data, dtype=np.float32).T)
    W = np.asarray(W, dtype=np.float32)
    b = np.asarray(b, dtype=np.float32)
    mask = np.zeros((128, 2048), np.float32)
    mask[np.arange(128), np.arange(128) + 1024] = 1.0
    ones = np.ones((128, 128), np.float32)
    in_maps = []
    for c in range(8):
        cols = np.concatenate(
            [embT[:, 1024 * s:1024 * (s + 1)] for s in SLOTS[c]], axis=1)
        in_maps.append({"embT": np.ascontiguousarray(cols), "W": W, "b": b,
                        "ones": ones, "mask": mask})
    return in_maps


def _combine(results):
    neg = np.zeros(8192, np.float64)
    pos = np.zeros(8192, np.float64)
    for c in range(8):
        S = SLOTS[c]
        rp = results[c]["rowpart"].astype(np.float64)
        cp = results[c]["colpart"].astype(np.float64)
        dg = results[c]["diagexp"].astype(np.float64).ravel()
        sl = [np.s_[1024 * s:1024 * (s + 1)] for s in S]
        neg[sl[0]] += rp[0] - dg          # diag block, self-sim removed
        neg[sl[0]] += rp[1]; neg[sl[1]] += cp[0]   # B1 (0,1)
        neg[sl[0]] += rp[2]; neg[sl[2]] += cp[1]   # B2 (0,2)
        neg[sl[1]] += rp[3]; neg[sl[3]] += cp[2]   # B3 (1,3)
        if c < 4:                                   # B4 (0,3) dedup: cores 0-3
            neg[sl[0]] += rp[4]; neg[sl[3]] += cp[3]
            ps = results[c]["possim"].astype(np.float64).ravel()
            pos[sl[0]] = ps
            pos[1024 * S[3]:1024 * (S[3] + 1)] = ps
    loss = -np.mean(10.0 * pos - np.log(neg))
    return np.float32(loss)


def run(embedded_data, W, b, trace=False):
    from concourse import bass_utils
    nc = _build()
    in_maps = _host_inputs(embedded_data, W, b)
    res = bass_utils.run_bass_kernel_spmd(nc, in_maps, core_ids=list(range(8)),
                                          trace=trace)
    return _combine(res.results), res


def kernel(embedded_data, W, b):
    loss, _ = run(embedded_data, W, b, trace=False)
    return np.asarray(loss, dtype=np.float32)



# revision 7
# speedup vs baseline: 1.7196x; 1.7196x over previous
"""NT-Xent contrastive loss on 8 Trainium2 NeuronCores (Bass/Tile), v2.

Same slab-cover strategy as v1 (no collectives): core c loads slabs
S_c = {c, c+1, c+2, c+4} (mod 8); every slab pair meets on some core, so
each of the 36 unique 1024x1024 sim blocks is computed once globally
(cores 0-3 dedup block B4 on host). v2 rebuilds the per-core kernel for
speed:

  * fp8(e4m3) inputs: embT pre-transposed AND pre-laid-out on host into
    the exact SBUF tile layout [128, slab, kchunk, row] so each slab DMA
    is 128 x 16 KiB contiguous descriptors (8 MiB/core vs 32 MiB in v1).
  * All heavy matmuls run fp8 DoubleRow (K=256 per instruction): head
    projection, sim blocks, and exp-column-sums (over mb-pair tiles,
    ones[128,2,1] x exp[128,2,512]).
  * L2-normalize: normsq via ones-matmul on bf16 squares; rsqrt done as
    Exp(-0.5*Ln(x)) on ScalarE (Rsqrt activation is banned; Ln+Exp share
    one ACT table set so there is no table thrashing); 1/norm broadcast
    via K=1 ones-row matmul; normalize multiply emits fp8 t_on directly.
  * exp tiles stored fp8e5 (max 57344 > e^10.5) in mb-PAIRED tiles
    [128, 2, 1024] so column sums can use DoubleRow.
  * Diagonal of the diag block: sim_ii as computed by the fp8 matmul is
    exactly sum_d u8[d,i]^2, so we recompute it cheaply (DVE square +
    ones-matmul + ACT exp) and subtract on host - no mask extraction.
  * pos term: elementwise product of fp8 slabs 0,3 + ones-matmul;
    log(pos) = 10*possim exactly (no exp).
  * PSUM budget = 8 banks exactly: head 2 (dh-sequential chains), sim 4
    (double-buffered [128,1024] + broadcast tiles share the pool),
    normsq 1, colsum 1 (nb-sequential chains over persistent exp pairs).
"""
import numpy as np
import ml_dtypes

SLOTS = [(c, (c + 1) % 8, (c + 2) % 8, (c + 4) % 8) for c in range(8)]
WSCALE = 32.0  # power of two; normalize() cancels it exactly

_CACHE = {}


def _build():
    if "nc" in _CACHE:
        return _CACHE["nc"]
    import concourse.bacc as bacc
    import concourse.tile as tile
    import concourse.mybir as mybir

    F32 = mybir.dt.float32
    BF16 = mybir.dt.bfloat16
    F8E4 = mybir.dt.float8e4
    F8E5 = mybir.dt.float8e5
    AF = mybir.ActivationFunctionType
    ALU = mybir.AluOpType
    DR = mybir.MatmulPerfMode.DoubleRow

    nc = bacc.Bacc("TRN2", num_devices=8, debug=False)
    a_emb = nc.dram_tensor("emb8", [128, 4, 16, 1024], F8E4,
                           kind="ExternalInput").ap()
    a_W = nc.dram_tensor("W8", [128, 16, 256], F8E4, kind="ExternalInput").ap()
    a_b = nc.dram_tensor("bS", [1, 256], BF16, kind="ExternalInput").ap()
    a_oc = nc.dram_tensor("ones_col", [128, 1], BF16, kind="ExternalInput").ap()
    a_or = nc.dram_tensor("ones_row", [1, 512], BF16, kind="ExternalInput").ap()
    a_o8 = nc.dram_tensor("ones8", [128, 32], F8E5, kind="ExternalInput").ap()
    o_rp = nc.dram_tensor("rowpart", [5, 1024], F32, kind="ExternalOutput").ap()
    o_cp = nc.dram_tensor("colpart", [4, 1024], F32, kind="ExternalOutput").ap()
    o_dg = nc.dram_tensor("diagexp", [1, 1024], F32, kind="ExternalOutput").ap()
    o_ps = nc.dram_tensor("possim", [1, 1024], F32, kind="ExternalOutput").ap()

    with tile.TileContext(nc) as tc:
        with tc.tile_pool(name="sb", bufs=1) as sb, \
             tc.tile_pool(name="emb", bufs=4) as embp, \
             tc.tile_pool(name="hp", bufs=2) as hp, \
             tc.tile_pool(name="sq", bufs=2) as sqp, \
             tc.tile_pool(name="rn", bufs=2) as rnp, \
             tc.tile_pool(name="ln", bufs=2) as lnp, \
             tc.tile_pool(name="expp", bufs=6) as expp, \
             tc.tile_pool(name="headp", bufs=2, space="PSUM") as headp, \
             tc.tile_pool(name="simp", bufs=2, space="PSUM") as simp, \
             tc.tile_pool(name="nsp", bufs=1, space="PSUM") as nsp, \
             tc.tile_pool(name="csp", bufs=1, space="PSUM") as csp:

            t_W = sb.tile([128, 16, 256], F8E4, name="t_W")
            nc.sync.dma_start(t_W[:], a_W[:])
            t_b = sb.tile([1, 256], BF16, name="t_b")
            nc.sync.dma_start(t_b[:], a_b[:])
            t_oc = sb.tile([128, 1], BF16, name="t_oc")
            nc.sync.dma_start(t_oc[:], a_oc[:])
            t_or = sb.tile([1, 512], BF16, name="t_or")
            nc.sync.dma_start(t_or[:], a_or[:])
            t_o8 = sb.tile([128, 2, 16], F8E5, name="t_o8")
            nc.sync.dma_start(t_o8[:], a_o8.rearrange("p (a o) -> p a o", o=16))

            # prefetch all four emb slabs
            t_e = []
            for k in range(4):
                te = embp.tile([128, 16, 1024], F8E4, name=f"t_e{k}", tag="emb")
                nc.sync.dma_start(te[:], a_emb[:, k, :, :])
                t_e.append(te)

            # persistent normalized slabs (fp8) and staging accumulators
            t_on = [sb.tile([128, 2, 1024], F8E4, name=f"t_on{k}")
                    for k in range(4)]
            rp_st = sb.tile([128, 5, 8], F32, name="rp_st")
            cp_st = sb.tile([1, 4096], F32, name="cp_st")
            dg_st = sb.tile([1, 1024], F32, name="dg_st")
            ps_st = sb.tile([1, 1024], F32, name="ps_st")

            def stage_a(k):
                """head projection + L2 normalize of slab k -> t_on[k]."""
                th = hp.tile([128, 2, 1024], BF16, name="t_h", tag="th")
                for h in range(2):
                    for dh in range(2):
                        ph = headp.tile([128, 512], F32, name="p_h", tag="head")
                        for j in range(8):
                            nc.tensor.matmul(
                                ph[:],
                                t_W[:, 2 * j:2 * j + 2,
                                    dh * 128:(dh + 1) * 128],
                                t_e[k][:, 2 * j:2 * j + 2,
                                       h * 512:(h + 1) * 512],
                                start=(j == 0), stop=False, perf_mode=DR)
                        # bias: + b[d] * ones_row  (K=1 bf16 matmul)
                        nc.tensor.matmul(
                            ph[:], t_b[0:1, dh * 128:(dh + 1) * 128],
                            t_or[0:1, :], start=False, stop=True)
                        nc.vector.tensor_copy(
                            th[:, dh, h * 512:(h + 1) * 512], ph[:])
                tsq = sqp.tile([128, 2, 1024], BF16, name="t_sq", tag="sq")
                nc.vector.tensor_tensor(tsq[:], th[:], th[:], ALU.mult)
                rn = rnp.tile([1, 1024], BF16, name="t_rn", tag="rn")
                for nb in range(2):
                    pns = nsp.tile([1, 512], F32, name="p_ns", tag="ns")
                    for dh in range(2):
                        nc.tensor.matmul(
                            pns[:], t_oc[:],
                            tsq[:, dh, nb * 512:(nb + 1) * 512],
                            start=(dh == 0), stop=(dh == 1))
                    tln = lnp.tile([1, 512], F32, name="t_ln", tag="ln")
                    nc.scalar.activation(tln[:], pns[:], AF.Ln)
                    nc.scalar.activation(rn[0:1, nb * 512:(nb + 1) * 512],
                                         tln[:], AF.Exp, scale=-0.5)
                pbc = simp.tile([128, 1024], F32, name="p_bc", tag="sim")
                for nb in range(2):
                    nc.tensor.matmul(pbc[:, nb * 512:(nb + 1) * 512],
                                     t_or[0:1, 0:128],
                                     rn[0:1, nb * 512:(nb + 1) * 512],
                                     start=True, stop=True)
                for dh in range(2):
                    nc.vector.tensor_tensor(t_on[k][:, dh, :], th[:, dh, :],
                                            pbc[:], ALU.mult)

            def block(bslot, a, bm):
                """sim block: rows = slab slot a, cols = slab slot bm."""
                texps = []
                for pair in range(4):
                    texp = expp.tile([128, 2, 1024], F8E5, name="t_exp",
                                     tag="exp")
                    texps.append(texp)
                    for half in range(2):
                        mb = 2 * pair + half
                        psim = simp.tile([128, 1024], F32, name="p_sim",
                                         tag="sim")
                        for nb in range(2):
                            nc.tensor.matmul(
                                psim[:, nb * 512:(nb + 1) * 512],
                                t_on[a][:, :, mb * 128:(mb + 1) * 128],
                                t_on[bm][:, :, nb * 512:(nb + 1) * 512],
                                start=True, stop=True, perf_mode=DR)
                        nc.scalar.activation(
                            texp[:, half, :], psim[:], AF.Exp, scale=10.0,
                            accum_out=rp_st[:, bslot, mb:mb + 1])
                if bslot > 0:
                    for nb in range(2):
                        pcs = csp.tile([1, 512], F32, name="p_cs", tag="cs")
                        for pair in range(4):
                            nc.tensor.matmul(
                                pcs[:], t_o8[:, :, 0:1],
                                texps[pair][:, :, nb * 512:(nb + 1) * 512],
                                start=(pair == 0), stop=(pair == 3),
                                perf_mode=DR)
                        nc.vector.tensor_copy(
                            cp_st[0:1, 1024 * (bslot - 1) + nb * 512:
                                  1024 * (bslot - 1) + (nb + 1) * 512],
                            pcs[:])

            def colreduce_exp(src8, dst, scale):
                """dst[1,1024] = f(sum_d src8[d,:]^2): f=exp(scale*x) or copy."""
                tq = sqp.tile([128, 2, 1024], BF16, name="t_q", tag="sq")
                nc.vector.tensor_tensor(tq[:], src8[0][:], src8[1][:],
                                        ALU.mult)
                for nb in range(2):
                    pr = nsp.tile([1, 512], F32, name="p_r", tag="ns")
                    for dh in range(2):
                        nc.tensor.matmul(
                            pr[:], t_oc[:], tq[:, dh, nb * 512:(nb + 1) * 512],
                            start=(dh == 0), stop=(dh == 1))
                    if scale is None:
                        nc.vector.tensor_copy(
                            dst[0:1, nb * 512:(nb + 1) * 512], pr[:])
                    else:
                        nc.scalar.activation(
                            dst[0:1, nb * 512:(nb + 1) * 512], pr[:],
                            AF.Exp, scale=scale)

            stage_a(0)
            # diag exp values: exp(10 * |u8_i|^2) == exp(10 * sim_ii)
            colreduce_exp((t_on[0], t_on[0]), dg_st, 10.0)
            block(0, 0, 0)
            stage_a(1)
            block(1, 0, 1)
            stage_a(2)
            block(2, 0, 2)
            stage_a(3)
            block(3, 1, 3)
            block(4, 0, 3)
            # pos: possim_i = sum_d u0[d,i]*u3[d,i]; host uses 10*possim
            colreduce_exp((t_on[0], t_on[3]), ps_st, None)

            for bslot in range(5):
                nc.sync.dma_start(
                    o_rp[bslot:bslot + 1, :].rearrange(
                        "o (m p) -> p (o m)", p=128),
                    rp_st[:, bslot, :])
            nc.sync.dma_start(o_cp.rearrange("a r -> (a r)")[None, :], cp_st[:])
            nc.sync.dma_start(o_dg[:], dg_st[:])
            nc.sync.dma_start(o_ps[:], ps_st[:])

    nc.compile()
    _CACHE["nc"] = nc
    return nc


def _host_inputs(embedded_data, W, b):
    emb = np.asarray(embedded_data, dtype=np.float32)      # [8192, 2048]
    W = np.asarray(W, dtype=np.float32)
    b = np.asarray(b, dtype=np.float32)
    # slab s tile layout: [128(p), 16(kc), 1024(r)], value = emb[r0+r, 128*kc+p]
    embT = np.ascontiguousarray(emb.T)                     # [2048, 8192]
    emb8 = embT.reshape(16, 128, 8192).transpose(1, 0, 2)  # [128, 16, 8192]
    emb8 = emb8.astype(ml_dtypes.float8_e4m3)
    W8 = (W * WSCALE).reshape(16, 128, 256).transpose(1, 0, 2)
    W8 = np.ascontiguousarray(W8).astype(ml_dtypes.float8_e4m3)
    bS = np.ascontiguousarray((b * WSCALE).reshape(1, 256)).astype(
        ml_dtypes.bfloat16)
    ones_col = np.ones((128, 1), ml_dtypes.bfloat16)
    ones_row = np.ones((1, 512), ml_dtypes.bfloat16)
    ones8 = np.ones((128, 32), ml_dtypes.float8_e5m2)
    in_maps = []
    for c in range(8):
        sl = np.stack([emb8[:, :, 1024 * s:1024 * (s + 1)] for s in SLOTS[c]],
                      axis=1)                              # [128, 4, 16, 1024]
        in_maps.append({"emb8": np.ascontiguousarray(sl), "W8": W8, "bS": bS,
                        "ones_col": ones_col, "ones_row": ones_row,
                        "ones8": ones8})
    return in_maps


def _combine(results):
    neg = np.zeros(8192, np.float64)
    pos = np.zeros(8192, np.float64)
    for c in range(8):
        S = SLOTS[c]
        rp = results[c]["rowpart"].astype(np.float64)
        cp = results[c]["colpart"].astype(np.float64)
        dg = results[c]["diagexp"].astype(np.float64).ravel()
        sl = [np.s_[1024 * s:1024 * (s + 1)] for s in S]
        neg[sl[0]] += rp[0] - dg          # diag block, self-sim removed
        neg[sl[0]] += rp[1]; neg[sl[1]] += cp[0]   # B1 (0,1)
        neg[sl[0]] += rp[2]; neg[sl[2]] += cp[1]   # B2 (0,2)
        neg[sl[1]] += rp[3]; neg[sl[3]] += cp[2]   # B3 (1,3)
        if c < 4:                                   # B4 (0,3) dedup: cores 0-3
            neg[sl[0]] += rp[4]; neg[sl[3]] += cp[3]
            ps = results[c]["possim"].astype(np.float64).ravel()
            pos[sl[0]] = ps
            pos[1024 * S[3]:1024 * (S[3] + 1)] = ps
    loss = -np.mean(10.0 * pos - np.log(neg))
    return np.float32(loss)


def run(embedded_data, W, b, trace=False):
    from concourse import bass_utils
    nc = _build()
    in_maps = _host_inputs(embedded_data, W, b)
    res = bass_utils.run_bass_kernel_spmd(nc, in_maps, core_ids=list(range(8)),
                                          trace=trace)
    return _combine(res.results), res


def kernel(embedded_data, W, b):
    loss, _ = run(embedded_data, W, b, trace=False)
    return np.asarray(loss, dtype=np.float32)


# revision 8
# speedup vs baseline: 2.0175x; 1.1733x over previous
"""NT-Xent contrastive loss on 8 Trainium2 NeuronCores (Bass/Tile), v2.

Same slab-cover strategy as v1 (no collectives): core c loads slabs
S_c = {c, c+1, c+2, c+4} (mod 8); every slab pair meets on some core, so
each of the 36 unique 1024x1024 sim blocks is computed once globally
(cores 0-3 dedup block B4 on host). v2 rebuilds the per-core kernel for
speed:

  * fp8(e4m3) inputs: embT pre-transposed AND pre-laid-out on host into
    the exact SBUF tile layout [128, slab, kchunk, row] so each slab DMA
    is 128 x 16 KiB contiguous descriptors (8 MiB/core vs 32 MiB in v1).
  * All heavy matmuls run fp8 DoubleRow (K=256 per instruction): head
    projection, sim blocks, and exp-column-sums (over mb-pair tiles,
    ones[128,2,1] x exp[128,2,512]).
  * L2-normalize: normsq via ones-matmul on bf16 squares; rsqrt done as
    Exp(-0.5*Ln(x)) on ScalarE (Rsqrt activation is banned; Ln+Exp share
    one ACT table set so there is no table thrashing); 1/norm broadcast
    via K=1 ones-row matmul; normalize multiply emits fp8 t_on directly.
  * exp tiles stored fp8e5 (max 57344 > e^10.5) in mb-PAIRED tiles
    [128, 2, 1024] so column sums can use DoubleRow.
  * Diagonal of the diag block: sim_ii as computed by the fp8 matmul is
    exactly sum_d u8[d,i]^2, so we recompute it cheaply (DVE square +
    ones-matmul + ACT exp) and subtract on host - no mask extraction.
  * pos term: elementwise product of fp8 slabs 0,3 + ones-matmul;
    log(pos) = 10*possim exactly (no exp).
  * PSUM budget = 8 banks exactly: head 2 (dh-sequential chains), sim 4
    (double-buffered [128,1024] + broadcast tiles share the pool),
    normsq 1, colsum 1 (nb-sequential chains over persistent exp pairs).
"""
import numpy as np
import ml_dtypes

SLOTS = [(c, (c + 1) % 8, (c + 2) % 8, (c + 4) % 8) for c in range(8)]
WSCALE = 32.0  # power of two; normalize() cancels it exactly

_CACHE = {}


def _build():
    if "nc" in _CACHE:
        return _CACHE["nc"]
    import concourse.bacc as bacc
    import concourse.tile as tile
    import concourse.mybir as mybir

    F32 = mybir.dt.float32
    BF16 = mybir.dt.bfloat16
    F8E4 = mybir.dt.float8e4
    F8E5 = mybir.dt.float8e5
    AF = mybir.ActivationFunctionType
    ALU = mybir.AluOpType
    DR = mybir.MatmulPerfMode.DoubleRow

    # Steer walrus act-table selection: keep Exp/Ln only in the combined
    # natural_log_exp_and_others set so the kernel needs ONE table load
    # instead of thrashing exp_and_others <-> natural_log (1.28us each).
    _orig_gat = bacc.get_activation_tables

    def _gat(arch):
        t = _orig_gat(arch)
        for name, fns in t.items():
            if name != "natural_log_exp_and_others":
                fns.discard(mybir.ActivationFunctionType.Exp)
                fns.discard(mybir.ActivationFunctionType.Ln)
        return t

    bacc.get_activation_tables = _gat

    nc = bacc.Bacc("TRN2", num_devices=8, debug=False)
    a_emb = nc.dram_tensor("emb8", [128, 4, 16, 1024], F8E4,
                           kind="ExternalInput").ap()
    a_W = nc.dram_tensor("W8", [128, 16, 256], F8E4, kind="ExternalInput").ap()
    a_b = nc.dram_tensor("bS", [1, 256], BF16, kind="ExternalInput").ap()
    a_oc = nc.dram_tensor("ones_col", [128, 1], BF16, kind="ExternalInput").ap()
    a_or = nc.dram_tensor("ones_row", [1, 512], BF16, kind="ExternalInput").ap()
    a_o8 = nc.dram_tensor("ones8", [128, 32], F8E5, kind="ExternalInput").ap()
    o_rp = nc.dram_tensor("rowpart", [128, 40], F32, kind="ExternalOutput").ap()
    o_cp = nc.dram_tensor("colpart", [4, 1024], F32, kind="ExternalOutput").ap()
    o_dg = nc.dram_tensor("diagexp", [1, 1024], F32, kind="ExternalOutput").ap()
    o_ps = nc.dram_tensor("possim", [1, 1024], F32, kind="ExternalOutput").ap()

    with tile.TileContext(nc) as tc:
        with tc.tile_pool(name="sb", bufs=1) as sb, \
             tc.tile_pool(name="emb", bufs=4) as embp, \
             tc.tile_pool(name="hp", bufs=2) as hp, \
             tc.tile_pool(name="sq", bufs=2) as sqp, \
             tc.tile_pool(name="rn", bufs=2) as rnp, \
             tc.tile_pool(name="ln", bufs=2) as lnp, \
             tc.tile_pool(name="expp", bufs=6) as expp, \
             tc.tile_pool(name="headp", bufs=2, space="PSUM") as headp, \
             tc.tile_pool(name="simp", bufs=2, space="PSUM") as simp, \
             tc.tile_pool(name="nsp", bufs=1, space="PSUM") as nsp, \
             tc.tile_pool(name="csp", bufs=1, space="PSUM") as csp:

            t_e = []
            te0 = embp.tile([128, 16, 1024], F8E4, name="t_e0", tag="emb")
            nc.sync.dma_start(te0[:], a_emb[:, 0, :, :])
            t_e.append(te0)
            t_W = sb.tile([128, 16, 256], F8E4, name="t_W")
            nc.sync.dma_start(t_W[:], a_W[:])
            t_b = sb.tile([1, 256], BF16, name="t_b")
            nc.sync.dma_start(t_b[:], a_b[:])
            t_oc = sb.tile([128, 1], BF16, name="t_oc")
            nc.sync.dma_start(t_oc[:], a_oc[:])
            t_or = sb.tile([1, 512], BF16, name="t_or")
            nc.sync.dma_start(t_or[:], a_or[:])
            t_o8 = sb.tile([128, 2, 16], F8E5, name="t_o8")
            nc.sync.dma_start(t_o8[:], a_o8.rearrange("p (a o) -> p a o", o=16))

            # prefetch remaining emb slabs
            for k in range(1, 4):
                te = embp.tile([128, 16, 1024], F8E4, name=f"t_e{k}", tag="emb")
                nc.sync.dma_start(te[:], a_emb[:, k, :, :])
                t_e.append(te)

            # persistent normalized slabs (fp8) and staging accumulators
            t_on = [sb.tile([128, 2, 1024], F8E4, name=f"t_on{k}")
                    for k in range(4)]
            rp_st = sb.tile([128, 5, 8], F32, name="rp_st")
            cp_st = sb.tile([1, 4096], F32, name="cp_st")
            dg_st = sb.tile([1, 1024], F32, name="dg_st")
            ps_st = sb.tile([1, 1024], F32, name="ps_st")

            def stage_a(k):
                """head projection + L2 normalize of slab k -> t_on[k]."""
                th = hp.tile([128, 2, 1024], BF16, name="t_h", tag="th")
                for h in range(2):
                    for dh in range(2):
                        ph = headp.tile([128, 512], F32, name="p_h", tag="head")
                        for j in range(8):
                            nc.tensor.matmul(
                                ph[:],
                                t_W[:, 2 * j:2 * j + 2,
                                    dh * 128:(dh + 1) * 128],
                                t_e[k][:, 2 * j:2 * j + 2,
                                       h * 512:(h + 1) * 512],
                                start=(j == 0), stop=False, perf_mode=DR)
                        # bias: + b[d] * ones_row  (K=1 bf16 matmul)
                        nc.tensor.matmul(
                            ph[:], t_b[0:1, dh * 128:(dh + 1) * 128],
                            t_or[0:1, :], start=False, stop=True)
                        nc.vector.tensor_copy(
                            th[:, dh, h * 512:(h + 1) * 512], ph[:])
                tsq = sqp.tile([128, 2, 1024], BF16, name="t_sq", tag="sq")
                nc.vector.tensor_tensor(tsq[:], th[:], th[:], ALU.mult)
                rn = rnp.tile([1, 1024], BF16, name="t_rn", tag="rn")
                for nb in range(2):
                    pns = nsp.tile([1, 512], F32, name="p_ns", tag="ns")
                    for dh in range(2):
                        nc.tensor.matmul(
                            pns[:], t_oc[:],
                            tsq[:, dh, nb * 512:(nb + 1) * 512],
                            start=(dh == 0), stop=(dh == 1))
                    tln = lnp.tile([1, 512], F32, name="t_ln", tag="ln")
                    nc.scalar.activation(tln[:], pns[:], AF.Ln)
                    nc.scalar.activation(rn[0:1, nb * 512:(nb + 1) * 512],
                                         tln[:], AF.Exp, scale=-0.5)
                pbc = simp.tile([128, 1024], F32, name="p_bc", tag="sim")
                for nb in range(2):
                    nc.tensor.matmul(pbc[:, nb * 512:(nb + 1) * 512],
                                     t_or[0:1, 0:128],
                                     rn[0:1, nb * 512:(nb + 1) * 512],
                                     start=True, stop=True)
                for dh in range(2):
                    nc.vector.tensor_tensor(t_on[k][:, dh, :], th[:, dh, :],
                                            pbc[:], ALU.mult)

            def block(bslot, a, bm):
                """sim block: rows = slab slot a, cols = slab slot bm."""
                texps = []
                for pair in range(4):
                    texp = expp.tile([128, 2, 1024], F8E5, name="t_exp",
                                     tag="exp")
                    texps.append(texp)
                    for half in range(2):
                        mb = 2 * pair + half
                        psim = simp.tile([128, 1024], F32, name="p_sim",
                                         tag="sim")
                        for nb in range(2):
                            nc.tensor.matmul(
                                psim[:, nb * 512:(nb + 1) * 512],
                                t_on[a][:, :, mb * 128:(mb + 1) * 128],
                                t_on[bm][:, :, nb * 512:(nb + 1) * 512],
                                start=True, stop=True, perf_mode=DR)
                        nc.scalar.activation(
                            texp[:, half, :], psim[:], AF.Exp, scale=10.0,
                            accum_out=rp_st[:, bslot, mb:mb + 1])
                if bslot > 0:
                    for nb in range(2):
                        pcs = csp.tile([1, 512], F32, name="p_cs", tag="cs")
                        for pair in range(4):
                            nc.tensor.matmul(
                                pcs[:], t_o8[:, :, 0:1],
                                texps[pair][:, :, nb * 512:(nb + 1) * 512],
                                start=(pair == 0), stop=(pair == 3),
                                perf_mode=DR)
                        nc.vector.tensor_copy(
                            cp_st[0:1, 1024 * (bslot - 1) + nb * 512:
                                  1024 * (bslot - 1) + (nb + 1) * 512],
                            pcs[:])

            def colreduce_exp(src8, dst, scale):
                """dst[1,1024] = f(sum_d src8[d,:]^2): f=exp(scale*x) or copy."""
                tq = sqp.tile([128, 2, 1024], BF16, name="t_q", tag="sq")
                nc.vector.tensor_tensor(tq[:], src8[0][:], src8[1][:],
                                        ALU.mult)
                for nb in range(2):
                    pr = nsp.tile([1, 512], F32, name="p_r", tag="ns")
                    for dh in range(2):
                        nc.tensor.matmul(
                            pr[:], t_oc[:], tq[:, dh, nb * 512:(nb + 1) * 512],
                            start=(dh == 0), stop=(dh == 1))
                    if scale is None:
                        nc.vector.tensor_copy(
                            dst[0:1, nb * 512:(nb + 1) * 512], pr[:])
                    else:
                        nc.scalar.activation(
                            dst[0:1, nb * 512:(nb + 1) * 512], pr[:],
                            AF.Exp, scale=scale)

            stage_a(0)
            # diag exp values: exp(10 * |u8_i|^2) == exp(10 * sim_ii)
            colreduce_exp((t_on[0], t_on[0]), dg_st, 10.0)
            block(0, 0, 0)
            stage_a(1)
            block(1, 0, 1)
            stage_a(2)
            block(2, 0, 2)
            stage_a(3)
            block(3, 1, 3)
            block(4, 0, 3)
            # pos: possim_i = sum_d u0[d,i]*u3[d,i]; host uses 10*possim
            colreduce_exp((t_on[0], t_on[3]), ps_st, None)

            nc.gpsimd.dma_start(o_rp[:],
                                rp_st[:].rearrange("p a b -> p (a b)"))
            nc.gpsimd.dma_start(o_cp.rearrange("a r -> (a r)")[None, :],
                                cp_st[:])
            nc.gpsimd.dma_start(o_dg[:], dg_st[:])
            nc.gpsimd.dma_start(o_ps[:], ps_st[:])

    try:
        nc.compile()
    finally:
        bacc.get_activation_tables = _orig_gat
    _CACHE["nc"] = nc
    return nc


def _host_inputs(embedded_data, W, b):
    emb = np.asarray(embedded_data, dtype=np.float32)      # [8192, 2048]
    W = np.asarray(W, dtype=np.float32)
    b = np.asarray(b, dtype=np.float32)
    # slab s tile layout: [128(p), 16(kc), 1024(r)], value = emb[r0+r, 128*kc+p]
    embT = np.ascontiguousarray(emb.T)                     # [2048, 8192]
    emb8 = embT.reshape(16, 128, 8192).transpose(1, 0, 2)  # [128, 16, 8192]
    emb8 = emb8.astype(ml_dtypes.float8_e4m3)
    W8 = (W * WSCALE).reshape(16, 128, 256).transpose(1, 0, 2)
    W8 = np.ascontiguousarray(W8).astype(ml_dtypes.float8_e4m3)
    bS = np.ascontiguousarray((b * WSCALE).reshape(1, 256)).astype(
        ml_dtypes.bfloat16)
    ones_col = np.ones((128, 1), ml_dtypes.bfloat16)
    ones_row = np.ones((1, 512), ml_dtypes.bfloat16)
    ones8 = np.ones((128, 32), ml_dtypes.float8_e5m2)
    in_maps = []
    for c in range(8):
        sl = np.stack([emb8[:, :, 1024 * s:1024 * (s + 1)] for s in SLOTS[c]],
                      axis=1)                              # [128, 4, 16, 1024]
        in_maps.append({"emb8": np.ascontiguousarray(sl), "W8": W8, "bS": bS,
                        "ones_col": ones_col, "ones_row": ones_row,
                        "ones8": ones8})
    return in_maps


def _combine(results):
    neg = np.zeros(8192, np.float64)
    pos = np.zeros(8192, np.float64)
    for c in range(8):
        S = SLOTS[c]
        rp = results[c]["rowpart"].astype(np.float64)
        rp = rp.reshape(128, 5, 8).transpose(1, 2, 0).reshape(5, 1024)
        cp = results[c]["colpart"].astype(np.float64)
        dg = results[c]["diagexp"].astype(np.float64).ravel()
        sl = [np.s_[1024 * s:1024 * (s + 1)] for s in S]
        neg[sl[0]] += rp[0] - dg          # diag block, self-sim removed
        neg[sl[0]] += rp[1]; neg[sl[1]] += cp[0]   # B1 (0,1)
        neg[sl[0]] += rp[2]; neg[sl[2]] += cp[1]   # B2 (0,2)
        neg[sl[1]] += rp[3]; neg[sl[3]] += cp[2]   # B3 (1,3)
        if c < 4:                                   # B4 (0,3) dedup: cores 0-3
            neg[sl[0]] += rp[4]; neg[sl[3]] += cp[3]
            ps = results[c]["possim"].astype(np.float64).ravel()
            pos[sl[0]] = ps
            pos[1024 * S[3]:1024 * (S[3] + 1)] = ps
    loss = -np.mean(10.0 * pos - np.log(neg))
    return np.float32(loss)


def run(embedded_data, W, b, trace=False):
    from concourse import bass_utils
    nc = _build()
    in_maps = _host_inputs(embedded_data, W, b)
    res = bass_utils.run_bass_kernel_spmd(nc, in_maps, core_ids=list(range(8)),
                                          trace=trace)
    return _combine(res.results), res


def kernel(embedded_data, W, b):
    loss, _ = run(embedded_data, W, b, trace=False)
    return np.asarray(loss, dtype=np.float32)
